# revision 8
# baseline (speedup 1.0000x reference)
"""OnlineTripletLoss Trainium2 kernel (8 NeuronCores, SPMD).

Strategy (label-space mining):
  pos_mask = positive_idxs[:, target_idx] is a column permutation of the raw
  mask. Instead of permuting the 16MB masks, permute the 2MB embedding once:
  g[l] = embedding[inv_target[l]].  Mining for anchor i then runs over label
  axis l with the raw (contiguous) masks:
      d2'[i,l] = C0 + ||e_i - g_l + eps||^2   (expanded, via PE matmul)
      hardest pos: max over l of d2'[i,l] * mp[i,l]        (mp in {0,1})
      hardest neg: min over l of d2'[i,l] * wn[i,l]        (wn in {1,200})
  The {1,200} trick keeps invalid entries out of the min because
  200*min(d2') > max(d2') always (d2' in [~31, ~1100]).
  Indices recovered with one max_index pass (exact f32 value match), then
  p/n rows are gathered by indirect DMA and ap/an/pn are recomputed exactly
  in f32 (avoids the winner's-curse bias of reading values off the noisy
  bf16-matmul d2).

Per core: 512 anchors x 4096 labels, 4 blocks of 128 anchors.
Outputs per core: masked per-anchor loss and validity; host sums and divides.
"""

import numpy as np
import ml_dtypes

import concourse.bass as bass
import concourse.mybir as mybir
import concourse.tile as tile
from concourse import bacc
from concourse.bass_utils import run_bass_kernel_spmd
import concourse.dve_ops as dve_ops
from concourse.dve_ops import DveOp
from concourse.dve_spec import Spec, Src0, Src1, maxx, lower, _has_src1
from concourse.dve_spec import C0 as DVE_C0, C2 as DVE_C2
from concourse.dve_uop import DveOpSpec

_OPNAME = "TT_MUL_RMAX_ANT"


def _ref_tt_mul_rmax(in0, in1, s0, s1, imm2):
    b = (in0.astype(np.float32) * in1 * np.float32(imm2)).astype(np.float32)
    mx = b.reshape(b.shape[0], -1).max(axis=-1, keepdims=True)
    mx = np.maximum(np.asarray(s0, np.float32), mx).astype(np.float32)
    return b, mx


def register_tt_mul_rmax():
    """Custom DVE op: out = in0*in1*imm2, accum_out = max(s0, row-max(out)).

    One DVE pass fuses the mask multiply with the max reduction (the stock
    TENSOR_TENSOR_REDUCE ISA opcode faults at runtime on this stack, and its
    custom-table twin only supports add-accum). Min mining uses imm2=-1.
    """
    if _OPNAME in dve_ops._SUB_OPCODE_FOR_NAME:
        for op in dve_ops.OPS:
            if op.name == _OPNAME:
                return op
    spec = Spec(body=Src0 * Src1 * DVE_C2, accum=maxx, accum_init=DVE_C0,
                reference=_ref_tt_mul_rmax)
    row = max(dve_ops._SUB_OPCODE_FOR_NAME.values()) + 1
    assert row < 0x20
    shas = {}
    for ver in ("v3", "v4"):
        try:
            s = DveOpSpec(name=_OPNAME, opcode=row, uops=lower(spec, ver=ver),
                          rd1_en=_has_src1(spec))
            shas[ver] = s.sha(ver)
        except Exception:
            pass
    op = DveOp(_OPNAME, spec, subdim=False, uops_sha=shas)
    dve_ops.OPS.append(op)
    dve_ops.CUSTOM_DVE_SPECS[_OPNAME] = spec
    dve_ops._SUB_OPCODE_FOR_NAME[_OPNAME] = row
    return op

B, D = 4096, 128
M = 8              # cores
BL = B // M        # 512 anchors per core
P = 128            # partition block
NB = BL // P       # 4 anchor blocks per core
CH = 512           # psum chunk (one bank of f32)
NCH = B // CH      # 8 chunks
EPS = 1e-6
C0 = 32.0
MARGIN = 1.0

F32 = mybir.dt.float32
BF16 = mybir.dt.bfloat16
U8 = mybir.dt.uint8
U32 = mybir.dt.uint32


def build_nc(debug: bool = False):
    ttr_op = register_tt_mul_rmax()
    nc = bacc.Bacc("TRN2", target_bir_lowering=False, debug=debug)

    eT = nc.dram_tensor("eT", [P, BL], BF16, kind="ExternalInput")      # -2*e_local^T
    gT = nc.dram_tensor("gT", [P, B], BF16, kind="ExternalInput")       # g^T
    cg = nc.dram_tensor("cg", [1, B], BF16, kind="ExternalInput")       # per-label const
    onesk = nc.dram_tensor("onesk", [1, P], BF16, kind="ExternalInput")
    arow = nc.dram_tensor("arow", [BL, 1], F32, kind="ExternalInput")   # per-anchor const
    el = nc.dram_tensor("el", [BL, D], F32, kind="ExternalInput")       # anchor rows f32
    gfull = nc.dram_tensor("gfull", [B, D], F32, kind="ExternalInput")  # gather source
    mp = nc.dram_tensor("mp", [BL, B], U8, kind="ExternalInput")        # pos mask {0,1}
    wn = nc.dram_tensor("wn", [BL, B], U8, kind="ExternalInput")        # neg weight {1,200}

    lossv = nc.dram_tensor("lossv", [BL, 1], F32, kind="ExternalOutput")
    vout = nc.dram_tensor("vout", [BL, 1], F32, kind="ExternalOutput")

    with tile.TileContext(nc) as tc:
        with (
            tc.tile_pool(name="singles", bufs=1) as singles,
            tc.tile_pool(name="masks", bufs=2) as maskpool,
            tc.tile_pool(name="d2", bufs=2) as d2pool,
            tc.tile_pool(name="vscr", bufs=1) as vpool,
            tc.tile_pool(name="psum", bufs=1, space="PSUM") as psumpool,
            tc.tile_pool(name="sm", bufs=2) as sm,
        ):
            eT_s = singles.tile([P, BL], BF16)
            nc.sync.dma_start(eT_s[:], eT[:])
            gT_s = singles.tile([P, B], BF16)
            nc.sync.dma_start(gT_s[:], gT[:])
            cg_s = singles.tile([1, B], BF16)
            nc.sync.dma_start(cg_s[:], cg[:])
            ones_s = singles.tile([1, P], BF16)
            nc.sync.dma_start(ones_s[:], onesk[:])
            eps_b = singles.tile([P, 1], F32)
            nc.vector.memset(eps_b[:], EPS)

            for b in range(NB):
                rs = b * P
                mp_b = maskpool.tile([P, B], U8, tag="mp")
                nc.sync.dma_start(mp_b[:], mp[rs:rs + P, :])
                wn_b = maskpool.tile([P, B], U8, tag="wn")
                nc.sync.dma_start(wn_b[:], wn[rs:rs + P, :])
                arow_b = sm.tile([P, 1], F32, tag="arow")
                nc.sync.dma_start(arow_b[:], arow[rs:rs + P, :])
                el_b = sm.tile([P, D], F32, tag="el")
                nc.sync.dma_start(el_b[:], el[rs:rs + P, :])

                psum = psumpool.tile([P, B], F32)
                for c in range(NCH):
                    cs = slice(c * CH, (c + 1) * CH)
                    nc.tensor.matmul(
                        psum[:, cs], lhsT=eT_s[:, rs:rs + P], rhs=gT_s[:, cs],
                        start=True, stop=False,
                    )
                    nc.tensor.matmul(
                        psum[:, cs], lhsT=ones_s[:1, :P], rhs=cg_s[:1, cs],
                        start=False, stop=True,
                    )

                # d2s = psum + arow (per-partition bias), PSUM -> SBUF f32
                d2s = d2pool.tile([P, B], F32)
                for c in range(NCH):
                    cs = slice(c * CH, (c + 1) * CH)
                    nc.scalar.activation(
                        d2s[:, cs], psum[:, cs],
                        mybir.ActivationFunctionType.Identity,
                        bias=arow_b[:, 0:1], scale=1.0,
                    )

                v = vpool.tile([P, B], F32)
                Mp = sm.tile([P, 1], F32, tag="Mp")
                nc.vector._custom_dve(ttr_op, out=v[:], in0=d2s[:], in1=mp_b[:],
                                      s0=0.0, s1=0.0, imm2=1.0, accum_out=Mp[:])
                Mneg = sm.tile([P, 1], F32, tag="Mneg")
                nc.vector._custom_dve(ttr_op, out=v[:], in0=d2s[:], in1=wn_b[:],
                                      s0=-1e30, s1=0.0, imm2=-1.0, accum_out=Mneg[:])
                Mn = sm.tile([P, 1], F32, tag="Mn")
                nc.vector.tensor_scalar(Mn[:], Mneg[:], -1.0, scalar2=None,
                                        op0=mybir.AluOpType.mult)

                inmax = sm.tile([P, 8], F32, tag="inmax")
                nc.vector.memset(inmax[:], -1.0)
                nc.vector.tensor_copy(inmax[:, 0:1], Mp[:])
                nc.vector.tensor_copy(inmax[:, 1:2], Mn[:])
                idx8 = sm.tile([P, 8], U32, tag="idx8")
                nc.vector.max_index(idx8[:], inmax[:], d2s[:])

                p_t = sm.tile([P, D], F32, tag="p_t")
                nc.gpsimd.indirect_dma_start(
                    out=p_t[:], out_offset=None, in_=gfull[:],
                    in_offset=bass.IndirectOffsetOnAxis(ap=idx8[:, 0:1], axis=0),
                )
                n_t = sm.tile([P, D], F32, tag="n_t")
                nc.gpsimd.indirect_dma_start(
                    out=n_t[:], out_offset=None, in_=gfull[:],
                    in_offset=bass.IndirectOffsetOnAxis(ap=idx8[:, 1:2], axis=0),
                )

                # exact f32 recompute: ap=||a-p+eps||, an=||a-n+eps||, pn=||p-n+eps||
                dif = sm.tile([P, D], F32, tag="dif")
                sq = sm.tile([P, D], F32, tag="sq")
                ap2 = sm.tile([P, 1], F32, tag="ap2")
                an2 = sm.tile([P, 1], F32, tag="an2")
                pn2 = sm.tile([P, 1], F32, tag="pn2")

                nc.vector.tensor_sub(dif[:], el_b[:], p_t[:])
                nc.scalar.activation(sq[:], dif[:],
                                     mybir.ActivationFunctionType.Square,
                                     bias=eps_b[:, 0:1], scale=1.0, accum_out=ap2[:])
                nc.vector.tensor_sub(dif[:], el_b[:], n_t[:])
                nc.scalar.activation(sq[:], dif[:],
                                     mybir.ActivationFunctionType.Square,
                                     bias=eps_b[:, 0:1], scale=1.0, accum_out=an2[:])
                nc.vector.tensor_sub(dif[:], p_t[:], n_t[:])
                nc.scalar.activation(sq[:], dif[:],
                                     mybir.ActivationFunctionType.Square,
                                     bias=eps_b[:, 0:1], scale=1.0, accum_out=pn2[:])

                ap = sm.tile([P, 1], F32, tag="ap")
                an = sm.tile([P, 1], F32, tag="an")
                pn = sm.tile([P, 1], F32, tag="pn")
                nc.scalar.activation(ap[:], ap2[:], mybir.ActivationFunctionType.Sqrt)
                nc.scalar.activation(an[:], an2[:], mybir.ActivationFunctionType.Sqrt)
                nc.scalar.activation(pn[:], pn2[:], mybir.ActivationFunctionType.Sqrt)

                vp = sm.tile([P, 1], F32, tag="vp")
                vn = sm.tile([P, 1], F32, tag="vn")
                valid = sm.tile([P, 1], F32, tag="valid")
                nc.vector.tensor_scalar(vp[:], Mp[:], 16.0, scalar2=None,
                                        op0=mybir.AluOpType.is_gt)
                nc.vector.tensor_scalar(vn[:], Mn[:], 3000.0, scalar2=None,
                                        op0=mybir.AluOpType.is_lt)
                nc.vector.tensor_mul(valid[:], vp[:], vn[:])

                mn2 = sm.tile([P, 1], F32, tag="mn2")
                nc.vector.tensor_tensor(out=mn2[:], in0=an[:], in1=pn[:],
                                        op=mybir.AluOpType.min)
                dff = sm.tile([P, 1], F32, tag="dff")
                nc.vector.tensor_sub(dff[:], ap[:], mn2[:])
                lossb = sm.tile([P, 1], F32, tag="lossb")
                nc.scalar.activation(lossb[:], dff[:],
                                     mybir.ActivationFunctionType.Relu,
                                     bias=MARGIN, scale=1.0)
                lout = sm.tile([P, 1], F32, tag="lout")
                nc.vector.tensor_mul(lout[:], lossb[:], valid[:])

                nc.sync.dma_start(lossv[rs:rs + P, :], lout[:])
                nc.sync.dma_start(vout[rs:rs + P, :], valid[:])

    nc.finalize()
    return nc


def make_in_maps(embedding, target_idx, positive_idxs, negative_idxs):
    e = np.asarray(embedding, np.float32)
    tid = np.asarray(target_idx, np.int64)
    pos = np.asarray(positive_idxs)
    neg = np.asarray(negative_idxs)

    inv = np.empty(B, np.int64)
    inv[tid] = np.arange(B)
    g = np.ascontiguousarray(e[inv])                       # [B, D] f32

    e64 = e.astype(np.float64)
    g64 = g.astype(np.float64)
    sq_a = (e64 * e64).sum(1)
    s_a = e64.sum(1)
    sq_g = (g64 * g64).sum(1)
    s_g = g64.sum(1)

    gT_bf = np.ascontiguousarray(g.T).astype(ml_dtypes.bfloat16)         # [D, B]
    cg_bf = np.asarray((sq_g - 2.0 * EPS * s_g)[None, :], ml_dtypes.bfloat16)
    ones_bf = np.ones((1, P), ml_dtypes.bfloat16)
    arow_full = np.asarray(sq_a + 2.0 * EPS * s_a + D * EPS * EPS + C0, np.float32)

    in_maps = []
    for m in range(M):
        r = slice(m * BL, (m + 1) * BL)
        in_maps.append({
            "eT": np.ascontiguousarray((-2.0 * e[r].T)).astype(ml_dtypes.bfloat16),
            "gT": gT_bf,
            "cg": cg_bf,
            "onesk": ones_bf,
            "arow": np.ascontiguousarray(arow_full[r][:, None]),
            "el": np.ascontiguousarray(e[r]),
            "gfull": g,
            "mp": np.ascontiguousarray(pos[r].astype(np.uint8)),
            "wn": np.ascontiguousarray(np.where(neg[r], 1, 200).astype(np.uint8)),
        })
    return in_maps


_NC_CACHE = {}


def kernel(embedding, target_idx, positive_idxs, negative_idxs):
    in_maps = make_in_maps(embedding, target_idx, positive_idxs, negative_idxs)
    if "nc" not in _NC_CACHE:
        _NC_CACHE["nc"] = build_nc(debug=False)
    nc = _NC_CACHE["nc"]
    res = run_bass_kernel_spmd(nc, in_maps, core_ids=list(range(M)))
    total_loss = np.float64(0.0)
    total_valid = np.float64(0.0)
    for r in res.results:
        total_loss += np.asarray(r["lossv"], np.float64).sum()
        total_valid += np.asarray(r["vout"], np.float64).sum()
    return np.float32(total_loss / max(total_valid, 1.0))


# revision 9
# speedup vs baseline: 1.0792x; 1.0792x over previous
"""OnlineTripletLoss Trainium2 kernel (8 NeuronCores, SPMD).

Strategy (label-space mining):
  pos_mask = positive_idxs[:, target_idx] is a column permutation of the raw
  mask. Instead of permuting the 16MB masks, permute the 2MB embedding once:
  g[l] = embedding[inv_target[l]].  Mining for anchor i then runs over label
  axis l with the raw (contiguous) masks:
      d2'[i,l] = C0 + ||e_i - g_l + eps||^2   (expanded, via PE matmul)
      hardest pos: max over l of d2'[i,l] * mp[i,l]        (mp in {0,1})
      hardest neg: min over l of d2'[i,l] * wn[i,l]        (wn in {1,200})
  The {1,200} trick keeps invalid entries out of the min because
  200*min(d2') > max(d2') always (d2' in [~31, ~1100]).
  Indices recovered with one max_index pass (exact f32 value match), then
  p/n rows are gathered by indirect DMA and ap/an/pn are recomputed exactly
  in f32 (avoids the winner's-curse bias of reading values off the noisy
  bf16-matmul d2).

Per core: 512 anchors x 4096 labels, 4 blocks of 128 anchors.
Outputs per core: masked per-anchor loss and validity; host sums and divides.
"""

import numpy as np
import ml_dtypes

import concourse.bass as bass
import concourse.mybir as mybir
import concourse.tile as tile
from concourse import bacc
from concourse.bass_utils import run_bass_kernel_spmd
import concourse.dve_ops as dve_ops
from concourse.dve_ops import DveOp
from concourse.dve_spec import Spec, Src0, Src1, maxx, lower, _has_src1
from concourse.dve_spec import C0 as DVE_C0, C2 as DVE_C2
from concourse.dve_uop import DveOpSpec

_OPNAME = "TT_MUL_RMAX_ANT"


def _ref_tt_mul_rmax(in0, in1, s0, s1, imm2):
    b = (in0.astype(np.float32) * in1 * np.float32(imm2)).astype(np.float32)
    mx = b.reshape(b.shape[0], -1).max(axis=-1, keepdims=True)
    mx = np.maximum(np.asarray(s0, np.float32), mx).astype(np.float32)
    return b, mx


def register_tt_mul_rmax():
    """Custom DVE op: out = in0*in1*imm2, accum_out = max(s0, row-max(out)).

    One DVE pass fuses the mask multiply with the max reduction (the stock
    TENSOR_TENSOR_REDUCE ISA opcode faults at runtime on this stack, and its
    custom-table twin only supports add-accum). Min mining uses imm2=-1.
    """
    if _OPNAME in dve_ops._SUB_OPCODE_FOR_NAME:
        for op in dve_ops.OPS:
            if op.name == _OPNAME:
                return op
    spec = Spec(body=Src0 * Src1 * DVE_C2, accum=maxx, accum_init=DVE_C0,
                reference=_ref_tt_mul_rmax)
    row = max(dve_ops._SUB_OPCODE_FOR_NAME.values()) + 1
    assert row < 0x20
    shas = {}
    for ver in ("v3", "v4"):
        try:
            s = DveOpSpec(name=_OPNAME, opcode=row, uops=lower(spec, ver=ver),
                          rd1_en=_has_src1(spec))
            shas[ver] = s.sha(ver)
        except Exception:
            pass
    op = DveOp(_OPNAME, spec, subdim=False, uops_sha=shas)
    dve_ops.OPS.append(op)
    dve_ops.CUSTOM_DVE_SPECS[_OPNAME] = spec
    dve_ops._SUB_OPCODE_FOR_NAME[_OPNAME] = row
    return op

B, D = 4096, 128
M = 8              # cores
BL = B // M        # 512 anchors per core
P = 128            # partition block
NB = BL // P       # 4 anchor blocks per core
CH = 512           # psum chunk (one bank of f32)
NCH = B // CH      # 8 chunks
EPS = 1e-6
C0 = 32.0
MARGIN = 1.0

F32 = mybir.dt.float32
BF16 = mybir.dt.bfloat16
U8 = mybir.dt.uint8
U32 = mybir.dt.uint32


def build_nc(debug: bool = False):
    ttr_op = register_tt_mul_rmax()
    nc = bacc.Bacc("TRN2", target_bir_lowering=False, debug=debug)

    eT = nc.dram_tensor("eT", [P, BL], BF16, kind="ExternalInput")      # -2*e_local^T
    gT = nc.dram_tensor("gT", [P, B], BF16, kind="ExternalInput")       # g^T
    cg = nc.dram_tensor("cg", [1, B], BF16, kind="ExternalInput")       # per-label const
    onesk = nc.dram_tensor("onesk", [1, P], BF16, kind="ExternalInput")
    arow = nc.dram_tensor("arow", [BL, 1], F32, kind="ExternalInput")   # per-anchor const
    el = nc.dram_tensor("el", [BL, D], F32, kind="ExternalInput")       # anchor rows f32
    gfull = nc.dram_tensor("gfull", [B, D], F32, kind="ExternalInput")  # gather source
    mp = nc.dram_tensor("mp", [BL, B], U8, kind="ExternalInput")        # pos mask {0,1}
    wn = nc.dram_tensor("wn", [BL, B], U8, kind="ExternalInput")        # neg weight {1,200}

    lossv = nc.dram_tensor("lossv", [BL, 1], F32, kind="ExternalOutput")
    vout = nc.dram_tensor("vout", [BL, 1], F32, kind="ExternalOutput")

    with tile.TileContext(nc) as tc:
        with (
            tc.tile_pool(name="singles", bufs=1) as singles,
            tc.tile_pool(name="masks", bufs=3) as maskpool,
            tc.tile_pool(name="d2", bufs=2) as d2pool,
            tc.tile_pool(name="vscr", bufs=1) as vpool,
            tc.tile_pool(name="psum", bufs=1, space="PSUM") as psumpool,
            tc.tile_pool(name="sm", bufs=1) as sm,
        ):
            eT_s = singles.tile([P, BL], BF16)
            nc.sync.dma_start(eT_s[:], eT[:])
            # chunked gT load so matmul chunk c starts as soon as its slice lands
            gT_s = singles.tile([P, B], BF16)
            for c in range(NCH):
                cs = slice(c * CH, (c + 1) * CH)
                nc.sync.dma_start(gT_s[:, cs], gT[:, cs])
            cg_s = singles.tile([1, B], BF16)
            nc.sync.dma_start(cg_s[:], cg[:])
            ones_s = singles.tile([1, P], BF16)
            nc.sync.dma_start(ones_s[:], onesk[:])
            eps_b = singles.tile([P, 1], F32)
            nc.vector.memset(eps_b[:], EPS)

            # batched per-anchor state: [128, NB(, D)] layouts, block = free col
            el_all = singles.tile([P, NB, D], F32)
            nc.sync.dma_start(el_all[:], el.rearrange("(b p) d -> p b d", b=NB))
            arow_all = singles.tile([P, NB], F32)
            nc.sync.dma_start(arow_all[:],
                              arow.rearrange("(b p) one -> p (b one)", b=NB))
            Mp_all = singles.tile([P, NB], F32)
            Mneg_all = singles.tile([P, NB], F32)
            idx_all = singles.tile([P, NB, 8], U32)
            p_all = singles.tile([P, NB, D], F32)
            n_all = singles.tile([P, NB, D], F32)
            inmax = singles.tile([P, 8], F32)
            nc.vector.memset(inmax[:], -1.0)

            for b in range(NB):
                rs = b * P
                mp_b = maskpool.tile([P, B], U8, tag="mp")
                nc.sync.dma_start(mp_b[:], mp[rs:rs + P, :])
                wn_b = maskpool.tile([P, B], U8, tag="wn")
                nc.sync.dma_start(wn_b[:], wn[rs:rs + P, :])

                psum = psumpool.tile([P, B], F32)
                for c in range(NCH):
                    cs = slice(c * CH, (c + 1) * CH)
                    nc.tensor.matmul(
                        psum[:, cs], lhsT=eT_s[:, rs:rs + P], rhs=gT_s[:, cs],
                        start=True, stop=False,
                    )
                    nc.tensor.matmul(
                        psum[:, cs], lhsT=ones_s[:1, :P], rhs=cg_s[:1, cs],
                        start=False, stop=True,
                    )

                # d2s = psum + arow (per-partition bias), PSUM -> SBUF f32
                d2s = d2pool.tile([P, B], F32)
                nc.scalar.activation(
                    d2s[:], psum[:],
                    mybir.ActivationFunctionType.Identity,
                    bias=arow_all[:, b:b + 1], scale=1.0,
                )

                v = vpool.tile([P, B], F32)
                nc.vector._custom_dve(ttr_op, out=v[:], in0=d2s[:], in1=mp_b[:],
                                      s0=0.0, s1=0.0, imm2=1.0,
                                      accum_out=Mp_all[:, b:b + 1])
                nc.vector._custom_dve(ttr_op, out=v[:], in0=d2s[:], in1=wn_b[:],
                                      s0=-1e30, s1=0.0, imm2=-1.0,
                                      accum_out=Mneg_all[:, b:b + 1])
                nc.vector.tensor_copy(inmax[:, 0:1], Mp_all[:, b:b + 1])
                nc.vector.tensor_scalar(inmax[:, 1:2], Mneg_all[:, b:b + 1], -1.0,
                                        scalar2=None, op0=mybir.AluOpType.mult)
                nc.vector.max_index(idx_all[:, b, :], inmax[:], d2s[:])

                nc.gpsimd.indirect_dma_start(
                    out=p_all[:, b, :], out_offset=None, in_=gfull[:],
                    in_offset=bass.IndirectOffsetOnAxis(ap=idx_all[:, b, 0:1], axis=0),
                )
                nc.gpsimd.indirect_dma_start(
                    out=n_all[:, b, :], out_offset=None, in_=gfull[:],
                    in_offset=bass.IndirectOffsetOnAxis(ap=idx_all[:, b, 1:2], axis=0),
                )

            # ---- batched tail over all NB blocks ----
            # exact f32: ap=||a-p+eps||, an=||a-n+eps||, pn=||p-n+eps||
            dif = sm.tile([P, NB, D], F32)
            sq = sm.tile([P, NB, D], F32)
            rt2 = sm.tile([P, 3 * NB], F32)   # [ap2 x NB | an2 x NB | pn2 x NB]
            for k, (x, y) in enumerate(((el_all, p_all), (el_all, n_all),
                                        (p_all, n_all))):
                nc.vector.tensor_sub(dif[:], x[:], y[:])
                nc.scalar.activation(sq[:], dif[:],
                                     mybir.ActivationFunctionType.Square,
                                     bias=eps_b[:, 0:1], scale=1.0)
                nc.vector.tensor_reduce(
                    out=rt2[:, k * NB:(k + 1) * NB], in_=sq[:],
                    axis=mybir.AxisListType.X, op=mybir.AluOpType.add)
            rt = sm.tile([P, 3 * NB], F32)
            nc.scalar.activation(rt[:], rt2[:], mybir.ActivationFunctionType.Sqrt)

            vp = sm.tile([P, NB], F32)
            vn = sm.tile([P, NB], F32)
            valid = sm.tile([P, NB], F32)
            nc.vector.tensor_scalar(vp[:], Mp_all[:], 16.0, scalar2=None,
                                    op0=mybir.AluOpType.is_gt)
            nc.vector.tensor_scalar(vn[:], Mneg_all[:], -3000.0, scalar2=None,
                                    op0=mybir.AluOpType.is_gt)
            nc.vector.tensor_mul(valid[:], vp[:], vn[:])

            mn2 = sm.tile([P, NB], F32)
            nc.vector.tensor_tensor(out=mn2[:], in0=rt[:, NB:2 * NB],
                                    in1=rt[:, 2 * NB:3 * NB],
                                    op=mybir.AluOpType.min)
            dff = sm.tile([P, NB], F32)
            nc.vector.tensor_sub(dff[:], rt[:, 0:NB], mn2[:])
            lossb = sm.tile([P, NB], F32)
            nc.scalar.activation(lossb[:], dff[:],
                                 mybir.ActivationFunctionType.Relu,
                                 bias=MARGIN, scale=1.0)
            lout = sm.tile([P, NB], F32)
            nc.vector.tensor_mul(lout[:], lossb[:], valid[:])

            nc.sync.dma_start(
                lossv.rearrange("(b p) one -> p (b one)", b=NB), lout[:])
            nc.sync.dma_start(
                vout.rearrange("(b p) one -> p (b one)", b=NB), valid[:])

    nc.finalize()
    return nc


def make_in_maps(embedding, target_idx, positive_idxs, negative_idxs):
    e = np.asarray(embedding, np.float32)
    tid = np.asarray(target_idx, np.int64)
    pos = np.asarray(positive_idxs)
    neg = np.asarray(negative_idxs)

    inv = np.empty(B, np.int64)
    inv[tid] = np.arange(B)
    g = np.ascontiguousarray(e[inv])                       # [B, D] f32

    e64 = e.astype(np.float64)
    g64 = g.astype(np.float64)
    sq_a = (e64 * e64).sum(1)
    s_a = e64.sum(1)
    sq_g = (g64 * g64).sum(1)
    s_g = g64.sum(1)

    gT_bf = np.ascontiguousarray(g.T).astype(ml_dtypes.bfloat16)         # [D, B]
    cg_bf = np.asarray((sq_g - 2.0 * EPS * s_g)[None, :], ml_dtypes.bfloat16)
    ones_bf = np.ones((1, P), ml_dtypes.bfloat16)
    arow_full = np.asarray(sq_a + 2.0 * EPS * s_a + D * EPS * EPS + C0, np.float32)

    in_maps = []
    for m in range(M):
        r = slice(m * BL, (m + 1) * BL)
        in_maps.append({
            "eT": np.ascontiguousarray((-2.0 * e[r].T)).astype(ml_dtypes.bfloat16),
            "gT": gT_bf,
            "cg": cg_bf,
            "onesk": ones_bf,
            "arow": np.ascontiguousarray(arow_full[r][:, None]),
            "el": np.ascontiguousarray(e[r]),
            "gfull": g,
            "mp": np.ascontiguousarray(pos[r].astype(np.uint8)),
            "wn": np.ascontiguousarray(np.where(neg[r], 1, 200).astype(np.uint8)),
        })
    return in_maps


_NC_CACHE = {}


def kernel(embedding, target_idx, positive_idxs, negative_idxs):
    in_maps = make_in_maps(embedding, target_idx, positive_idxs, negative_idxs)
    if "nc" not in _NC_CACHE:
        _NC_CACHE["nc"] = build_nc(debug=False)
    nc = _NC_CACHE["nc"]
    res = run_bass_kernel_spmd(nc, in_maps, core_ids=list(range(M)))
    total_loss = np.float64(0.0)
    total_valid = np.float64(0.0)
    for r in res.results:
        total_loss += np.asarray(r["lossv"], np.float64).sum()
        total_valid += np.asarray(r["vout"], np.float64).sum()
    return np.float32(total_loss / max(total_valid, 1.0))


# revision 17
# speedup vs baseline: 1.2339x; 1.1433x over previous
"""OnlineTripletLoss Trainium2 kernel (8 NeuronCores, SPMD).

Strategy (label-space mining):
  pos_mask = positive_idxs[:, target_idx] is a column permutation of the raw
  mask. Instead of permuting the 16MB masks, permute the 2MB embedding once:
  g[l] = embedding[inv_target[l]].  Mining for anchor i then runs over label
  axis l with the raw (contiguous) masks:
      d2'[i,l] = C0 + ||e_i - g_l + eps||^2   (expanded, via PE matmul)
      hardest pos: max over l of d2'[i,l] * mp[i,l]        (mp in {0,1})
      hardest neg: min over l of d2'[i,l] * wn[i,l]        (wn in {1,200})
  The {1,200} trick keeps invalid entries out of the min because
  200*min(d2') > max(d2') always (d2' in [~31, ~1100]).
  Indices recovered with one max_index pass (exact f32 value match), then
  p/n rows are gathered by indirect DMA and ap/an/pn are recomputed exactly
  in f32 (avoids the winner's-curse bias of reading values off the noisy
  bf16-matmul d2).

Per core: 512 anchors x 4096 labels, 4 blocks of 128 anchors.
Outputs per core: masked per-anchor loss and validity; host sums and divides.
"""

import numpy as np
import ml_dtypes

import concourse.bass as bass
import concourse.mybir as mybir
import concourse.tile as tile
from concourse import bacc
from concourse.bass_utils import run_bass_kernel_spmd
import concourse.dve_ops as dve_ops
from concourse.dve_ops import DveOp
from concourse.dve_spec import Spec, Src0, Src1, maxx, lower, _has_src1
from concourse.dve_spec import C0 as DVE_C0, C2 as DVE_C2
from concourse.dve_uop import DveOpSpec

_OPNAME = "TT_MUL_RMAX_ANT"


def _ref_tt_mul_rmax(in0, in1, s0, s1, imm2):
    b = (in0.astype(np.float32) * in1 * np.float32(imm2)).astype(np.float32)
    mx = b.reshape(b.shape[0], -1).max(axis=-1, keepdims=True)
    mx = np.maximum(np.asarray(s0, np.float32), mx).astype(np.float32)
    return b, mx


def register_tt_mul_rmax():
    """Custom DVE op: out = in0*in1*imm2, accum_out = max(s0, row-max(out)).

    One DVE pass fuses the mask multiply with the max reduction (the stock
    TENSOR_TENSOR_REDUCE ISA opcode faults at runtime on this stack, and its
    custom-table twin only supports add-accum). Min mining uses imm2=-1.
    """
    if _OPNAME in dve_ops._SUB_OPCODE_FOR_NAME:
        for op in dve_ops.OPS:
            if op.name == _OPNAME:
                return op
    spec = Spec(body=Src0 * Src1 * DVE_C2, accum=maxx, accum_init=DVE_C0,
                reference=_ref_tt_mul_rmax)
    row = max(dve_ops._SUB_OPCODE_FOR_NAME.values()) + 1
    assert row < 0x20
    shas = {}
    for ver in ("v3", "v4"):
        try:
            s = DveOpSpec(name=_OPNAME, opcode=row, uops=lower(spec, ver=ver),
                          rd1_en=_has_src1(spec))
            shas[ver] = s.sha(ver)
        except Exception:
            pass
    op = DveOp(_OPNAME, spec, subdim=False, uops_sha=shas)
    dve_ops.OPS.append(op)
    dve_ops.CUSTOM_DVE_SPECS[_OPNAME] = spec
    dve_ops._SUB_OPCODE_FOR_NAME[_OPNAME] = row
    return op

B, D = 4096, 128
M = 8              # cores
BL = B // M        # 512 anchors per core
P = 128            # partition block
NB = BL // P       # 4 anchor blocks per core
CH = 512           # psum chunk (one bank of f32)
NCH = B // CH      # 8 chunks
EPS = 1e-6
C0 = 32.0
MARGIN = 1.0

F32 = mybir.dt.float32
BF16 = mybir.dt.bfloat16
U8 = mybir.dt.uint8
U32 = mybir.dt.uint32


def build_nc(debug: bool = False):
    ttr_op = register_tt_mul_rmax()
    nc = bacc.Bacc("TRN2", target_bir_lowering=False, debug=debug)

    eT = nc.dram_tensor("eT", [P, BL], BF16, kind="ExternalInput")      # -2*e_local^T
    gT = nc.dram_tensor("gT", [P, B], BF16, kind="ExternalInput")       # g^T
    cg = nc.dram_tensor("cg", [1, B], BF16, kind="ExternalInput")       # per-label const
    onesk = nc.dram_tensor("onesk", [1, P], BF16, kind="ExternalInput")
    arow = nc.dram_tensor("arow", [P, NB], F32, kind="ExternalInput")   # per-anchor const
    el = nc.dram_tensor("el", [P, NB, D], F32, kind="ExternalInput")    # anchor rows f32
    gfull = nc.dram_tensor("gfull", [B, D], F32, kind="ExternalInput")  # gather source
    mp = nc.dram_tensor("mp", [BL, B], U8, kind="ExternalInput")        # pos mask {0,1}
    wn = nc.dram_tensor("wn", [BL, B], U8, kind="ExternalInput")        # neg weight {1,200}

    lossv = nc.dram_tensor("lossv", [P, NB], F32, kind="ExternalOutput")
    vout = nc.dram_tensor("vout", [P, NB], F32, kind="ExternalOutput")

    with tile.TileContext(nc) as tc:
        with (
            tc.tile_pool(name="singles", bufs=1) as singles,
            tc.tile_pool(name="masks", bufs=3) as maskpool,
            tc.tile_pool(name="d2", bufs=2) as d2pool,
            tc.tile_pool(name="vscr", bufs=1) as vpool,
            tc.tile_pool(name="psum", bufs=1, space="PSUM") as psumpool,
            tc.tile_pool(name="sm", bufs=1) as sm,
        ):
            eT_s = singles.tile([P, BL], BF16)
            nc.sync.dma_start(eT_s[:], eT[:])
            # chunked gT load so matmul chunk c starts as soon as its slice lands
            gT_s = singles.tile([P, B], BF16)
            for c in range(NCH):
                cs = slice(c * CH, (c + 1) * CH)
                nc.sync.dma_start(gT_s[:, cs], gT[:, cs])
            cg_s = singles.tile([1, B], BF16)
            nc.sync.dma_start(cg_s[:], cg[:])
            ones_s = singles.tile([1, P], BF16)
            nc.sync.dma_start(ones_s[:], onesk[:])
            eps_b = singles.tile([P, 1], F32)
            nc.vector.memset(eps_b[:], EPS)

            # batched per-anchor state (host pre-arranged contiguous):
            # loaded off the busy sync queue so they land immediately
            el_all = singles.tile([P, NB, D], F32)
            nc.scalar.dma_start(el_all[:], el[:])
            arow_all = singles.tile([P, NB], F32)
            nc.scalar.dma_start(arow_all[:], arow[:])
            Mp_all = singles.tile([P, NB], F32)
            Mneg_all = singles.tile([P, NB], F32)
            idx_all = singles.tile([P, NB, 8], U32)
            p_all = singles.tile([P, NB, D], F32)
            n_all = singles.tile([P, NB, D], F32)
            inmax = singles.tile([P, 8], F32)
            nc.vector.memset(inmax[:], -1.0)

            for b in range(NB):
                rs = b * P
                mp_b = maskpool.tile([P, B], U8, tag="mp")
                nc.sync.dma_start(mp_b[:], mp[rs:rs + P, :])
                wn_b = maskpool.tile([P, B], U8, tag="wn")
                nc.sync.dma_start(wn_b[:], wn[rs:rs + P, :])

                psum = psumpool.tile([P, B], F32)
                for c in range(NCH):
                    cs = slice(c * CH, (c + 1) * CH)
                    nc.tensor.matmul(
                        psum[:, cs], lhsT=eT_s[:, rs:rs + P], rhs=gT_s[:, cs],
                        start=True, stop=False,
                    )
                    nc.tensor.matmul(
                        psum[:, cs], lhsT=ones_s[:1, :P], rhs=cg_s[:1, cs],
                        start=False, stop=True,
                    )

                # d2s = psum + arow (per-partition bias), PSUM -> SBUF f32
                # per-chunk so chunk c converts as soon as its matmul lands
                d2s = d2pool.tile([P, B], F32)
                for c in range(NCH):
                    cs = slice(c * CH, (c + 1) * CH)
                    nc.scalar.activation(
                        d2s[:, cs], psum[:, cs],
                        mybir.ActivationFunctionType.Identity,
                        bias=arow_all[:, b:b + 1], scale=1.0,
                    )

                v = vpool.tile([P, B], F32)
                nc.vector._custom_dve(ttr_op, out=v[:], in0=d2s[:], in1=mp_b[:],
                                      s0=0.0, s1=0.0, imm2=1.0,
                                      accum_out=Mp_all[:, b:b + 1])
                nc.vector._custom_dve(ttr_op, out=v[:], in0=d2s[:], in1=wn_b[:],
                                      s0=-1e30, s1=0.0, imm2=-1.0,
                                      accum_out=Mneg_all[:, b:b + 1])
                nc.vector.tensor_copy(inmax[:, 0:1], Mp_all[:, b:b + 1])
                nc.vector.tensor_scalar(inmax[:, 1:2], Mneg_all[:, b:b + 1], -1.0,
                                        scalar2=None, op0=mybir.AluOpType.mult)
                nc.vector.max_index(idx_all[:, b, :], inmax[:], d2s[:])

                nc.gpsimd.indirect_dma_start(
                    out=p_all[:, b, :], out_offset=None, in_=gfull[:],
                    in_offset=bass.IndirectOffsetOnAxis(ap=idx_all[:, b, 0:1], axis=0),
                )
                nc.gpsimd.indirect_dma_start(
                    out=n_all[:, b, :], out_offset=None, in_=gfull[:],
                    in_offset=bass.IndirectOffsetOnAxis(ap=idx_all[:, b, 1:2], axis=0),
                )

            # ---- batched tail ----
            # exact f32: ap=||a-p+eps||, an=||a-n+eps||, pn=||p-n+eps||
            # split: blocks [0, NB-1) first (their gathers are long done while
            # block NB-1's gathers are still in flight), then the last block
            dif = sm.tile([P, NB, D], F32)
            sq = sm.tile([P, NB, D], F32)
            rt2 = sm.tile([P, 3 * NB], F32)   # [ap2 x NB | an2 x NB | pn2 x NB]
            pairs = ((el_all, p_all), (el_all, n_all), (p_all, n_all))
            for lo, hi in ((0, NB - 1), (NB - 1, NB)):
                n = hi - lo
                for k, (x, y) in enumerate(pairs):
                    nc.vector.tensor_sub(dif[:, lo:hi, :], x[:, lo:hi, :],
                                         y[:, lo:hi, :])
                    nc.scalar.activation(sq[:, lo:hi, :], dif[:, lo:hi, :],
                                         mybir.ActivationFunctionType.Square,
                                         bias=eps_b[:, 0:1], scale=1.0)
                    nc.vector.tensor_reduce(
                        out=rt2[:, k * NB + lo:k * NB + hi],
                        in_=sq[:, lo:hi, :],
                        axis=mybir.AxisListType.X, op=mybir.AluOpType.add)
            rt = sm.tile([P, 3 * NB], F32)
            nc.scalar.activation(rt[:], rt2[:], mybir.ActivationFunctionType.Sqrt)

            vp = sm.tile([P, NB], F32)
            vn = sm.tile([P, NB], F32)
            valid = sm.tile([P, NB], F32)
            nc.vector.tensor_scalar(vp[:], Mp_all[:], 16.0, scalar2=None,
                                    op0=mybir.AluOpType.is_gt)
            nc.vector.tensor_scalar(vn[:], Mneg_all[:], -3000.0, scalar2=None,
                                    op0=mybir.AluOpType.is_gt)
            nc.vector.tensor_mul(valid[:], vp[:], vn[:])

            mn2 = sm.tile([P, NB], F32)
            nc.vector.tensor_tensor(out=mn2[:], in0=rt[:, NB:2 * NB],
                                    in1=rt[:, 2 * NB:3 * NB],
                                    op=mybir.AluOpType.min)
            dff = sm.tile([P, NB], F32)
            nc.vector.tensor_sub(dff[:], rt[:, 0:NB], mn2[:])
            lossb = sm.tile([P, NB], F32)
            nc.scalar.activation(lossb[:], dff[:],
                                 mybir.ActivationFunctionType.Relu,
                                 bias=MARGIN, scale=1.0)
            lout = sm.tile([P, NB], F32)
            nc.vector.tensor_mul(lout[:], lossb[:], valid[:])

            nc.sync.dma_start(lossv[:], lout[:])
            nc.sync.dma_start(vout[:], valid[:])

    nc.finalize()
    return nc


def make_in_maps(embedding, target_idx, positive_idxs, negative_idxs):
    e = np.asarray(embedding, np.float32)
    tid = np.asarray(target_idx, np.int64)
    pos = np.asarray(positive_idxs)
    neg = np.asarray(negative_idxs)

    inv = np.empty(B, np.int64)
    inv[tid] = np.arange(B)
    g = np.ascontiguousarray(e[inv])                       # [B, D] f32

    e64 = e.astype(np.float64)
    g64 = g.astype(np.float64)
    sq_a = (e64 * e64).sum(1)
    s_a = e64.sum(1)
    sq_g = (g64 * g64).sum(1)
    s_g = g64.sum(1)

    gT_bf = np.ascontiguousarray(g.T).astype(ml_dtypes.bfloat16)         # [D, B]
    cg_bf = np.asarray((sq_g - 2.0 * EPS * s_g)[None, :], ml_dtypes.bfloat16)
    ones_bf = np.ones((1, P), ml_dtypes.bfloat16)
    arow_full = np.asarray(sq_a + 2.0 * EPS * s_a + D * EPS * EPS + C0, np.float32)

    in_maps = []
    for m in range(M):
        r = slice(m * BL, (m + 1) * BL)
        # [P, NB(, D)] layouts: block index on the free axis
        el3 = np.ascontiguousarray(
            e[r].reshape(NB, P, D).transpose(1, 0, 2))
        arow2 = np.ascontiguousarray(arow_full[r].reshape(NB, P).T)
        in_maps.append({
            "eT": np.ascontiguousarray((-2.0 * e[r].T)).astype(ml_dtypes.bfloat16),
            "gT": gT_bf,
            "cg": cg_bf,
            "onesk": ones_bf,
            "arow": arow2,
            "el": el3,
            "gfull": g,
            "mp": np.ascontiguousarray(pos[r].astype(np.uint8)),
            "wn": np.ascontiguousarray(np.where(neg[r], 1, 200).astype(np.uint8)),
        })
    return in_maps


_NC_CACHE = {}


def kernel(embedding, target_idx, positive_idxs, negative_idxs):
    in_maps = make_in_maps(embedding, target_idx, positive_idxs, negative_idxs)
    if "nc" not in _NC_CACHE:
        _NC_CACHE["nc"] = build_nc(debug=False)
    nc = _NC_CACHE["nc"]
    res = run_bass_kernel_spmd(nc, in_maps, core_ids=list(range(M)))
    total_loss = np.float64(0.0)
    total_valid = np.float64(0.0)
    for r in res.results:
        total_loss += np.asarray(r["lossv"], np.float64).sum()
        total_valid += np.asarray(r["vout"], np.float64).sum()
    return np.float32(total_loss / max(total_valid, 1.0))


# revision 21
# speedup vs baseline: 1.2883x; 1.0441x over previous
"""OnlineTripletLoss Trainium2 kernel (8 NeuronCores, SPMD).

Strategy (label-space mining):
  pos_mask = positive_idxs[:, target_idx] is a column permutation of the raw
  mask. Instead of permuting the 16MB masks, permute the 2MB embedding once:
  g[l] = embedding[inv_target[l]].  Mining for anchor i then runs over label
  axis l with the raw (contiguous) masks:
      d2'[i,l] = C0 + ||e_i - g_l + eps||^2   (expanded, via PE matmul)
      hardest pos: max over l of d2'[i,l] * mp[i,l]        (mp in {0,1})
      hardest neg: min over l of d2'[i,l] * wn[i,l]        (wn in {1,200})
  The {1,200} trick keeps invalid entries out of the min because
  200*min(d2') > max(d2') always (d2' in [~31, ~1100]).
  Indices recovered with one max_index pass (exact f32 value match), then
  p/n rows are gathered by indirect DMA and ap/an/pn are recomputed exactly
  in f32 (avoids the winner's-curse bias of reading values off the noisy
  bf16-matmul d2).

Per core: 512 anchors x 4096 labels, 4 blocks of 128 anchors.
Outputs per core: masked per-anchor loss and validity; host sums and divides.
"""

import numpy as np
import ml_dtypes

import concourse.bass as bass
import concourse.mybir as mybir
import concourse.tile as tile
from concourse import bacc
from concourse.bass_utils import run_bass_kernel_spmd
import concourse.dve_ops as dve_ops
from concourse.dve_ops import DveOp
from concourse.dve_spec import Spec, Src0, Src1, maxx, lower, _has_src1
from concourse.dve_spec import C0 as DVE_C0, C2 as DVE_C2
from concourse.dve_uop import DveOpSpec

_OPNAME = "TT_MUL_RMAX_ANT"


def _ref_tt_mul_rmax(in0, in1, s0, s1, imm2):
    b = (in0.astype(np.float32) * in1 * np.float32(imm2)).astype(np.float32)
    mx = b.reshape(b.shape[0], -1).max(axis=-1, keepdims=True)
    mx = np.maximum(np.asarray(s0, np.float32), mx).astype(np.float32)
    return b, mx


def register_tt_mul_rmax():
    """Custom DVE op: out = in0*in1*imm2, accum_out = max(s0, row-max(out)).

    One DVE pass fuses the mask multiply with the max reduction (the stock
    TENSOR_TENSOR_REDUCE ISA opcode faults at runtime on this stack, and its
    custom-table twin only supports add-accum). Min mining uses imm2=-1.
    """
    if _OPNAME in dve_ops._SUB_OPCODE_FOR_NAME:
        for op in dve_ops.OPS:
            if op.name == _OPNAME:
                return op
    spec = Spec(body=Src0 * Src1 * DVE_C2, accum=maxx, accum_init=DVE_C0,
                reference=_ref_tt_mul_rmax)
    row = max(dve_ops._SUB_OPCODE_FOR_NAME.values()) + 1
    assert row < 0x20
    shas = {}
    for ver in ("v3", "v4"):
        try:
            s = DveOpSpec(name=_OPNAME, opcode=row, uops=lower(spec, ver=ver),
                          rd1_en=_has_src1(spec))
            shas[ver] = s.sha(ver)
        except Exception:
            pass
    op = DveOp(_OPNAME, spec, subdim=False, uops_sha=shas)
    dve_ops.OPS.append(op)
    dve_ops.CUSTOM_DVE_SPECS[_OPNAME] = spec
    dve_ops._SUB_OPCODE_FOR_NAME[_OPNAME] = row
    return op

B, D = 4096, 128
M = 8              # cores
BL = B // M        # 512 anchors per core
P = 128            # partition block
NB = BL // P       # 4 anchor blocks per core
CH = 512           # psum chunk (one bank of f32)
NCH = B // CH      # 8 chunks
EPS = 1e-6
C0 = 32.0
MARGIN = 1.0

F32 = mybir.dt.float32
BF16 = mybir.dt.bfloat16
U8 = mybir.dt.uint8
U32 = mybir.dt.uint32


def build_nc(debug: bool = False):
    ttr_op = register_tt_mul_rmax()
    nc = bacc.Bacc("TRN2", target_bir_lowering=False, debug=debug)

    eT = nc.dram_tensor("eT", [P, BL], BF16, kind="ExternalInput")      # -2*e_local^T
    gT = nc.dram_tensor("gT", [P, B], BF16, kind="ExternalInput")       # g^T
    cg = nc.dram_tensor("cg", [1, B], BF16, kind="ExternalInput")       # per-label const
    onesk = nc.dram_tensor("onesk", [1, P], BF16, kind="ExternalInput")
    arow = nc.dram_tensor("arow", [P, NB], F32, kind="ExternalInput")   # per-anchor const
    el = nc.dram_tensor("el", [P, NB, D], F32, kind="ExternalInput")    # anchor rows f32
    gfull = nc.dram_tensor("gfull", [B, D], F32, kind="ExternalInput")  # gather source
    mp = nc.dram_tensor("mp", [BL, B], U8, kind="ExternalInput")        # pos mask {0,1}
    wn = nc.dram_tensor("wn", [BL, B], U8, kind="ExternalInput")        # neg weight {1,200}

    lossv = nc.dram_tensor("lossv", [P, NB], F32, kind="ExternalOutput")
    vout = nc.dram_tensor("vout", [P, NB], F32, kind="ExternalOutput")

    with tile.TileContext(nc) as tc:
        with (
            tc.tile_pool(name="singles", bufs=1) as singles,
            tc.tile_pool(name="masks", bufs=3) as maskpool,
            tc.tile_pool(name="d2", bufs=2) as d2pool,
            tc.tile_pool(name="vscr", bufs=1) as vpool,
            tc.tile_pool(name="psum", bufs=1, space="PSUM") as psumpool,
            tc.tile_pool(name="sm", bufs=1) as sm,
        ):
            eT_s = singles.tile([P, BL], BF16)
            nc.sync.dma_start(eT_s[:], eT[:])
            # chunked gT load so matmul chunk c starts as soon as its slice lands
            gT_s = singles.tile([P, B], BF16)
            for c in range(NCH):
                cs = slice(c * CH, (c + 1) * CH)
                nc.sync.dma_start(gT_s[:, cs], gT[:, cs])
            cg_s = singles.tile([1, B], BF16)
            nc.sync.dma_start(cg_s[:], cg[:])
            ones_s = singles.tile([1, P], BF16)
            nc.sync.dma_start(ones_s[:], onesk[:])
            eps_b = singles.tile([P, 1], F32)
            nc.vector.memset(eps_b[:], EPS)
            # touch Sqrt/Square/Relu once so ACT's table swap lands in the
            # fill shadow instead of the tail
            warm = singles.tile([P, 1], F32)
            nc.scalar.activation(warm[:], eps_b[:],
                                 mybir.ActivationFunctionType.Square)
            nc.scalar.activation(warm[:], warm[:],
                                 mybir.ActivationFunctionType.Sqrt)
            nc.scalar.activation(warm[:], warm[:],
                                 mybir.ActivationFunctionType.Relu)

            # batched per-anchor state (host pre-arranged contiguous):
            # loaded off the busy sync queue so they land immediately
            el_all = singles.tile([P, NB, D], F32)
            nc.scalar.dma_start(el_all[:], el[:])
            arow_all = singles.tile([P, NB], F32)
            nc.scalar.dma_start(arow_all[:], arow[:])
            Mp_all = singles.tile([P, NB], F32)
            Mneg_all = singles.tile([P, NB], F32)
            idx_all = singles.tile([P, NB, 8], U32)
            p_all = singles.tile([P, NB, D], F32)
            n_all = singles.tile([P, NB, D], F32)
            inmax = singles.tile([P, 8], F32)
            nc.vector.memset(inmax[:], -1.0)

            for b in range(NB):
                rs = b * P
                mp_b = maskpool.tile([P, B], U8, tag="mp")
                nc.sync.dma_start(mp_b[:], mp[rs:rs + P, :])
                wn_b = maskpool.tile([P, B], U8, tag="wn")
                nc.sync.dma_start(wn_b[:], wn[rs:rs + P, :])

                # grouped by lhsT so LDWEIGHTS isn't reloaded per chunk
                psum = psumpool.tile([P, B], F32)
                for c in range(NCH):
                    cs = slice(c * CH, (c + 1) * CH)
                    nc.tensor.matmul(
                        psum[:, cs], lhsT=eT_s[:, rs:rs + P], rhs=gT_s[:, cs],
                        start=True, stop=False,
                    )
                for c in range(NCH):
                    cs = slice(c * CH, (c + 1) * CH)
                    nc.tensor.matmul(
                        psum[:, cs], lhsT=ones_s[:1, :P], rhs=cg_s[:1, cs],
                        start=False, stop=True,
                    )

                # d2s = psum + arow (per-partition bias), PSUM -> SBUF f32
                # per-chunk so chunk c converts as soon as its matmul lands
                d2s = d2pool.tile([P, B], F32)
                for c in range(NCH):
                    cs = slice(c * CH, (c + 1) * CH)
                    nc.scalar.activation(
                        d2s[:, cs], psum[:, cs],
                        mybir.ActivationFunctionType.Identity,
                        bias=arow_all[:, b:b + 1], scale=1.0,
                    )

                v = vpool.tile([P, B], F32)
                nc.vector._custom_dve(ttr_op, out=v[:], in0=d2s[:], in1=mp_b[:],
                                      s0=0.0, s1=0.0, imm2=1.0,
                                      accum_out=Mp_all[:, b:b + 1])
                nc.vector._custom_dve(ttr_op, out=v[:], in0=d2s[:], in1=wn_b[:],
                                      s0=-1e30, s1=0.0, imm2=-1.0,
                                      accum_out=Mneg_all[:, b:b + 1])
                nc.vector.tensor_copy(inmax[:, 0:1], Mp_all[:, b:b + 1])
                nc.vector.tensor_scalar(inmax[:, 1:2], Mneg_all[:, b:b + 1], -1.0,
                                        scalar2=None, op0=mybir.AluOpType.mult)
                nc.vector.max_index(idx_all[:, b, :], inmax[:], d2s[:])

                nc.gpsimd.indirect_dma_start(
                    out=p_all[:, b, :], out_offset=None, in_=gfull[:],
                    in_offset=bass.IndirectOffsetOnAxis(ap=idx_all[:, b, 0:1], axis=0),
                )
                nc.gpsimd.indirect_dma_start(
                    out=n_all[:, b, :], out_offset=None, in_=gfull[:],
                    in_offset=bass.IndirectOffsetOnAxis(ap=idx_all[:, b, 1:2], axis=0),
                )

            # ---- batched tail ----
            # exact f32: ap=||a-p+eps||, an=||a-n+eps||, pn=||p-n+eps||
            # split: blocks [0, NB-1) first (their gathers are long done while
            # block NB-1's gathers are still in flight), then the last block
            # validity first: depends only on the TTR accums, fills the DVE
            # stream while the last block's gathers are in flight
            vp = sm.tile([P, NB], F32)
            vn = sm.tile([P, NB], F32)
            valid = sm.tile([P, NB], F32)
            nc.vector.tensor_scalar(vp[:], Mp_all[:], 16.0, scalar2=None,
                                    op0=mybir.AluOpType.is_gt)
            nc.vector.tensor_scalar(vn[:], Mneg_all[:], -3000.0, scalar2=None,
                                    op0=mybir.AluOpType.is_gt)
            nc.vector.tensor_mul(valid[:], vp[:], vn[:])

            dif = sm.tile([P, NB, D], F32)
            sq = sm.tile([P, NB, D], F32)
            rt2 = sm.tile([P, 3 * NB], F32)   # [ap2 x NB | an2 x NB | pn2 x NB]
            pairs = ((el_all, p_all), (el_all, n_all), (p_all, n_all))
            for lo, hi in ((0, NB - 1), (NB - 1, NB)):
                n = hi - lo
                for k, (x, y) in enumerate(pairs):
                    nc.vector.tensor_sub(dif[:, lo:hi, :], x[:, lo:hi, :],
                                         y[:, lo:hi, :])
                    nc.scalar.activation(sq[:, lo:hi, :], dif[:, lo:hi, :],
                                         mybir.ActivationFunctionType.Square,
                                         bias=eps_b[:, 0:1], scale=1.0)
                    nc.vector.tensor_reduce(
                        out=rt2[:, k * NB + lo:k * NB + hi],
                        in_=sq[:, lo:hi, :],
                        axis=mybir.AxisListType.X, op=mybir.AluOpType.add)
            rt = sm.tile([P, 3 * NB], F32)
            nc.scalar.activation(rt[:], rt2[:], mybir.ActivationFunctionType.Sqrt)

            mn2 = sm.tile([P, NB], F32)
            nc.vector.tensor_tensor(out=mn2[:], in0=rt[:, NB:2 * NB],
                                    in1=rt[:, 2 * NB:3 * NB],
                                    op=mybir.AluOpType.min)
            dff = sm.tile([P, NB], F32)
            nc.vector.tensor_sub(dff[:], rt[:, 0:NB], mn2[:])
            lossb = sm.tile([P, NB], F32)
            nc.scalar.activation(lossb[:], dff[:],
                                 mybir.ActivationFunctionType.Relu,
                                 bias=MARGIN, scale=1.0)
            lout = sm.tile([P, NB], F32)
            nc.vector.tensor_mul(lout[:], lossb[:], valid[:])

            nc.sync.dma_start(lossv[:], lout[:])
            nc.sync.dma_start(vout[:], valid[:])

    nc.finalize()
    return nc


def make_in_maps(embedding, target_idx, positive_idxs, negative_idxs):
    e = np.asarray(embedding, np.float32)
    tid = np.asarray(target_idx, np.int64)
    pos = np.asarray(positive_idxs)
    neg = np.asarray(negative_idxs)

    inv = np.empty(B, np.int64)
    inv[tid] = np.arange(B)
    g = np.ascontiguousarray(e[inv])                       # [B, D] f32

    e64 = e.astype(np.float64)
    g64 = g.astype(np.float64)
    sq_a = (e64 * e64).sum(1)
    s_a = e64.sum(1)
    sq_g = (g64 * g64).sum(1)
    s_g = g64.sum(1)

    gT_bf = np.ascontiguousarray(g.T).astype(ml_dtypes.bfloat16)         # [D, B]
    cg_bf = np.asarray((sq_g - 2.0 * EPS * s_g)[None, :], ml_dtypes.bfloat16)
    ones_bf = np.ones((1, P), ml_dtypes.bfloat16)
    arow_full = np.asarray(sq_a + 2.0 * EPS * s_a + D * EPS * EPS + C0, np.float32)

    in_maps = []
    for m in range(M):
        r = slice(m * BL, (m + 1) * BL)
        # [P, NB(, D)] layouts: block index on the free axis
        el3 = np.ascontiguousarray(
            e[r].reshape(NB, P, D).transpose(1, 0, 2))
        arow2 = np.ascontiguousarray(arow_full[r].reshape(NB, P).T)
        in_maps.append({
            "eT": np.ascontiguousarray((-2.0 * e[r].T)).astype(ml_dtypes.bfloat16),
            "gT": gT_bf,
            "cg": cg_bf,
            "onesk": ones_bf,
            "arow": arow2,
            "el": el3,
            "gfull": g,
            "mp": np.ascontiguousarray(pos[r].astype(np.uint8)),
            "wn": np.ascontiguousarray(np.where(neg[r], 1, 200).astype(np.uint8)),
        })
    return in_maps


_NC_CACHE = {}


def kernel(embedding, target_idx, positive_idxs, negative_idxs):
    in_maps = make_in_maps(embedding, target_idx, positive_idxs, negative_idxs)
    if "nc" not in _NC_CACHE:
        _NC_CACHE["nc"] = build_nc(debug=False)
    nc = _NC_CACHE["nc"]
    res = run_bass_kernel_spmd(nc, in_maps, core_ids=list(range(M)))
    total_loss = np.float64(0.0)
    total_valid = np.float64(0.0)
    for r in res.results:
        total_loss += np.asarray(r["lossv"], np.float64).sum()
        total_valid += np.asarray(r["vout"], np.float64).sum()
    return np.float32(total_loss / max(total_valid, 1.0))
